# revision 18
# baseline (speedup 1.0000x reference)
"""AudioCondTransformerEncoderLayer on 8 Trainium2 NeuronCores.

v3 strategy (TM=TA=512, B=32, D=1024, H=16, DFF=4096, 4 batch elems/core):
  - Data-parallel over batch across 8 cores; per-core per-b pipeline with
    cross-b overlap via pooled buffers.
  - fp8(e4m3) DoubleRow matmuls (2 k-chunks per instruction, ~1.7x bf16 on
    HW) for all projections (q/k/v/out, both attentions) and attnV; FFN
    layers fp8 per build flags (numerics-gated). Weights pre-scaled by
    pow2 per-tensor factors on host; descale via ACT drain `scale=` or
    folded into the stt residual drains (inv-scale columns live in ppb).
  - fp16 (not bf16) for the residual stream, q/k, LN math and exp-bias
    tiles: same PE/DVE/ACT cost as bf16, 4x less rounding noise.
  - Cross-attention bias: host-precomputed exp(bias) tiles, multiplied
    into the fp8 exp tiles on DVE (frees the PE identity-matmul adds).
    Banded: tk-chunk pairs (0,1)/(2,3) share 384-wide tq union windows;
    eb is zero-padded outside each chunk's true band so DoubleRow attnV
    over the union window is exact.
  - Softmax denominator via ones column in v (65-col attnV stationary);
    reciprocal on DVE, broadcast across partitions via PE outer product.
  - LayerNorm: partition sums via ones-column fp16 matmuls, variance via
    fused stt, mean/invstd broadcast via PE outer products; apply on DVE.
  - SA attention is ACT-bound (exp): CA k-proj + CA v-proj groups are
    interleaved into the SA hp loop to keep the PE fed; next-b v_proj is
    interleaved into CA attention the same way.
  - PSUM: scores 2x2-bank + mm 3x1 + LN-sums 1 = 8 banks.
"""

import os

import numpy as np

# ---------------------------------------------------------------------------
# Problem constants
# ---------------------------------------------------------------------------
D = 1024
H = 16
HD = 64
TM = 512
TA = 512
B = 32
DFF = 4096
NCORES = 8
BPC = B // NCORES          # batch elems per core
SIGMA = 4.0
BW = 2.0
LN_EPS = 1e-5
KD = D // 128              # 8 d-chunks
KF = DFF // 128            # 32 ff-chunks
TCH = TM // 128            # 4 token chunks

# banded CA: true tq band per tk-chunk, within 384-wide union windows
TQR = [(0, 256), (0, 384), (128, 384), (256, 256)]
U0 = [0, 0, 128, 128]      # union window start per chunk
UW = 384                   # union window width

_CACHE = {}


# ---------------------------------------------------------------------------
# Walrus workaround: this container's walrus build rejects >1 sync-wait per
# instruction. Split excess waits onto preceding same-engine NOPs, and move
# the tail drain's waits onto SP NOPs.
# ---------------------------------------------------------------------------
def _install_patches():
    if _CACHE.get("patched"):
        return
    import concourse.mybir as mybir
    import concourse.tile as tile
    import concourse.tile_utils as tile_utils
    from concourse.vector_clock import ScopedClock

    tile_utils.max_sbuf_usage = 208 * 1024

    _orig_commit = tile.TileContext._commit_instruction

    def _split_commit(self, inst, lazy_reg_writes=True):
        si = inst.sync_info
        if (
            si is not None
            and len(si.on_wait) > 1
            and inst.engine != mybir.EngineType.Unassigned
        ):
            waits = list(si.on_wait)
            inst.sync_info = mybir.SyncInfo(
                on_wait=waits[:1], on_update=list(si.on_update)
            )
            for w in waits[1:]:
                nop = mybir.InstNoOp(
                    name=self.nc.get_next_instruction_name(),
                    ins=[],
                    outs=[],
                    engine=inst.engine,
                    sync_info=mybir.SyncInfo(on_wait=[w], on_update=[]),
                )
                nop.debug = inst.debug
                _orig_commit(self, nop, lazy_reg_writes=False)
        return _orig_commit(self, inst, lazy_reg_writes=lazy_reg_writes)

    tile.TileContext._commit_instruction = _split_commit

    def _patched_drain_and_barrier(self, tick_clock, wait_clock):
        carrier = self.nc.sync.nop(nofuse=True)
        wait_clock.add_sem_waits(
            carrier.ins, ScopedClock({None: tick_clock.global_clock})
        )
        si = carrier.ins.sync_info
        if si is not None and len(si.on_wait) > 1:
            waits = list(si.on_wait)
            carrier.ins.sync_info = mybir.SyncInfo(
                on_wait=waits[:1], on_update=list(si.on_update)
            )
            for w in waits[1:]:
                extra = self.nc.sync.nop(nofuse=True)
                extra.ins.sync_info = mybir.SyncInfo(on_wait=[w], on_update=[])
        self.nc.sync.drain()
        self.nc.all_engine_barrier()
        popped = self.nc._tile_sem_poison_stack.pop()
        assert popped is self._sem_poison
        self.nc.clear_and_free_semaphores(list(self.sems.allocated().values()))
        self.nc.all_engine_barrier()

    tile.TileContext._drain_and_barrier = _patched_drain_and_barrier
    _CACHE["patched"] = True


# ---------------------------------------------------------------------------
# Device module
# ---------------------------------------------------------------------------
def _build_module(ln_id, bz, lin1_fp8, lin2_fp8, attnv_dr=True, proj_dr=True, nofill=False):
    """ln_id: LN affine is identity; bz: all GEMM biases are zero."""
    from contextlib import ExitStack

    import concourse.bass as bass
    import concourse.mybir as mybir
    import concourse.tile as tile

    f32 = mybir.dt.float32
    f32r = mybir.dt.float32r
    f16 = mybir.dt.float16
    fp8 = mybir.dt.float8e4
    AF = mybir.ActivationFunctionType
    OP = mybir.AluOpType
    DR = mybir.MatmulPerfMode.DoubleRow

    nc = bass.Bass()

    def din(name, shape, dt):
        return nc.dram_tensor(name, shape, dt, kind="ExternalInput")

    xin16 = din("xin16", (BPC, KD, 128, TM), f16)
    xin8 = din("xin8", (BPC, KD, 128, TM), fp8)
    ain8 = din("ain8", (BPC, KD, 128, TA), fp8)
    wqk_sa = din("wqk_sa", (16, 128, KD, 128), fp8)
    wv_sa = din("wv_sa", (2, 128, KD, 512), fp8)
    wo_sa = din("wo_sa", (8, 128, KD, 128), fp8)
    wqk_ca = din("wqk_ca", (16, 128, KD, 128), fp8)
    wv_ca = din("wv_ca", (2, 128, KD, 512), fp8)
    wo_ca = din("wo_ca", (8, 128, KD, 128), fp8)
    w1 = din("w1", (KF, 128, KD, 128), fp8 if lin1_fp8 else f16)
    w2 = din("w2", (8, 128, KF, 128), fp8 if lin2_fp8 else f16)
    # per-partition bias/gain/inv-scale columns:
    # [bqk_sa(16) bo_sa(8) bqk_ca(16) bo_ca(8) b1(32) b2(8)
    #  n1g n1b ncg ncb n2g n2b (6x8)
    #  inv: qk_sa v_sa o_sa qk_ca v_ca o_ca lin1 lin2 (8x1)] = 144 cols
    ppb = din("ppb", (128, 144), f32)
    eb = din("eb", (128, 4, UW), f16)   # exp(bias) per tk-chunk, 0-padded
    onescol = din("onescol", (128, 1), f32)
    onesrow = din("onesrow", (1, 128), f32)

    out = nc.dram_tensor("out", (BPC, KD, 128, TM), f16, kind="ExternalOutput")

    with tile.TileContext(nc) as tc, ExitStack() as ctx:
        cpool = ctx.enter_context(tc.tile_pool(name="consts", bufs=1))
        # fp16 stream tensors [128, KD, 512] = 8KB/part
        a16 = ctx.enter_context(tc.tile_pool(name="a16", bufs=7))
        # fp8 stream tensors [128, KD, 512] = 4KB/part
        a8p = ctx.enter_context(tc.tile_pool(name="a8p", bufs=6))
        vp = ctx.enter_context(tc.tile_pool(name="vp", bufs=3))       # v fp8
        hqp = ctx.enter_context(
            tc.tile_pool(name="hqp", bufs=2 if lin2_fp8 else 1))      # h
        expp = ctx.enter_context(tc.tile_pool(name="expp", bufs=8))   # e fp8
        wp = ctx.enter_context(tc.tile_pool(name="wqkp", bufs=8))     # 1KB w
        w16p = ctx.enter_context(tc.tile_pool(name="w16p", bufs=4))   # 2KB w1
        wvp = ctx.enter_context(tc.tile_pool(name="wvp", bufs=2))     # 4KB wv
        w2p = ctx.enter_context(tc.tile_pool(name="w2p", bufs=2))     # 4KB w2
        mip = ctx.enter_context(tc.tile_pool(name="mip", bufs=2))
        smp = ctx.enter_context(tc.tile_pool(name="small", bufs=2))
        smrp = ctx.enter_context(tc.tile_pool(name="smallr", bufs=2))
        mbp = ctx.enter_context(tc.tile_pool(name="mbp", bufs=2))     # miB
        bcp = ctx.enter_context(tc.tile_pool(name="bcast", bufs=3))   # bcs
        tmpp = ctx.enter_context(tc.tile_pool(name="tmp", bufs=3))
        sqp = ctx.enter_context(tc.tile_pool(name="sq", bufs=4))
        scrp = ctx.enter_context(tc.tile_pool(name="scratch", bufs=2))
        # psum: scorep 2x2-bank + mmp 3 + pss 1 = 8 banks
        scorep = ctx.enter_context(
            tc.tile_pool(name="scorep", bufs=2, space="PSUM"))
        mmp = ctx.enter_context(tc.tile_pool(name="mmp", bufs=3, space="PSUM"))
        pss = ctx.enter_context(tc.tile_pool(name="pss", bufs=1, space="PSUM"))

        # --- constants -----------------------------------------------------
        ones_c16 = cpool.tile([128, 1], f16, name="ones_c16")
        ones_c32 = cpool.tile([128, 1], f32, name="ones_c32")
        nc.sync.dma_start(ones_c32[:], onescol[:, :])
        nc.vector.tensor_copy(ones_c16[:], ones_c32[:])
        ones_r = cpool.tile([1, 128], f32r, name="ones_r")
        nc.sync.dma_start(ones_r[:], onesrow[:, :].bitcast(f32r))
        ebt = cpool.tile([128, 4, UW], f16, name="ebt")
        nc.sync.dma_start(ebt[:], eb[:, :, :])
        eps_t = cpool.tile([1, 1], f32, name="eps_t")
        nc.vector.memset(eps_t[:], LN_EPS)

        ppb_t = cpool.tile([128, 144], f32, name="ppb_t")
        nc.sync.dma_start(ppb_t[:], ppb[:, :])
        _off = [0]

        def pp_view(n):
            o = _off[0]
            _off[0] += n
            return ppb_t[:, o:o + n]

        bqk_sa_t = pp_view(16)
        bo_sa_t = pp_view(8)
        bqk_ca_t = pp_view(16)
        bo_ca_t = pp_view(8)
        b1_t = pp_view(KF)
        b2_t = pp_view(8)
        n1g_t, n1b_t = pp_view(8), pp_view(8)
        ncg_t, ncb_t = pp_view(8), pp_view(8)
        n2g_t, n2b_t = pp_view(8), pp_view(8)
        inv_qk_sa = pp_view(1)
        inv_v_sa = pp_view(1)
        inv_o_sa = pp_view(1)
        inv_qk_ca = pp_view(1)
        inv_v_ca = pp_view(1)
        inv_o_ca = pp_view(1)
        inv_1 = pp_view(1)
        inv_2 = pp_view(1)

        def t16(name):
            return a16.tile([128, KD, TM], f16, tag="a16", name=name)

        def t8(name):
            return a8p.tile([128, KD, TM], fp8, tag="a8", name=name)

        # --- helpers -------------------------------------------------------
        def ln(y, g_t, b_t, dst16, dst8):
            """LayerNorm over the feature axis of y [128,KD,T] fp16."""
            ps_s = pss.tile([1, TM], f32, tag="sps", name="ps_s")
            for k in range(KD):
                nc.tensor.matmul(ps_s[:], ones_c16[:], y[:, k],
                                 start=(k == 0), stop=(k == KD - 1))
            mi = mip.tile([1, 2, TM], f32r, tag="mi", name="mi")
            with nc.allow_low_precision(reason="ln mean f32r for bcast mm"):
                nc.scalar.mul(mi[:, 0], ps_s[:], 1.0 / D)
            miB = mbp.tile([128, 2, TM], f16, tag="bcl", name="miB")
            bmu = mmp.tile([128, TM], f32, tag="mm", name="bmu")
            nc.tensor.matmul(bmu[:], ones_r[:], mi[:, 0], start=True, stop=True)
            nc.scalar.copy(miB[:, 0], bmu[:])
            ps_q = pss.tile([1, TM], f32, tag="sps", name="ps_q")
            for k in range(KD):
                sq = sqp.tile([128, TM], f16, tag="sq", name="sq")
                if k % 2 == 0:
                    nc.vector.tensor_tensor(sq[:], y[:, k], y[:, k], OP.mult)
                else:
                    nc.scalar.activation(sq[:], y[:, k], AF.Square)
                nc.tensor.matmul(ps_q[:], ones_c16[:], sq[:],
                                 start=(k == 0), stop=(k == KD - 1))
            m2 = smp.tile([1, TM], f32, tag="sm", name="m2")
            nc.vector.tensor_tensor(m2[:], mi[:, 0].bitcast(f32),
                                    mi[:, 0].bitcast(f32), OP.mult)
            var = smp.tile([1, TM], f32, tag="sm", name="var")
            nc.vector.scalar_tensor_tensor(var[:], ps_q[:], 1.0 / D, m2[:],
                                           OP.mult, OP.subtract)
            sd = smp.tile([1, TM], f32, tag="sm", name="sd")
            nc.scalar.activation(sd[:], var[:], AF.Sqrt, bias=eps_t[:])
            with nc.allow_low_precision(reason="ln invstd recip"):
                nc.vector.reciprocal(mi[:, 1], sd[:])
            biv = mmp.tile([128, TM], f32, tag="mm", name="biv")
            nc.tensor.matmul(biv[:], ones_r[:], mi[:, 1], start=True, stop=True)
            nc.scalar.copy(miB[:, 1], biv[:])
            for k in range(KD):
                t1 = tmpp.tile([128, TM], f16, tag="t1", name="t1")
                nc.vector.tensor_tensor(t1[:], y[:, k], miB[:, 0], OP.subtract)
                if ln_id:
                    nc.vector.tensor_tensor(dst16[:, k], t1[:], miB[:, 1],
                                            OP.mult)
                else:
                    t2 = tmpp.tile([128, TM], f16, tag="t1", name="t2")
                    nc.vector.scalar_tensor_tensor(
                        t2[:], t1[:], g_t[:, k:k + 1], miB[:, 1],
                        OP.mult, OP.mult)
                    nc.vector.tensor_scalar_add(dst16[:, k], t2[:],
                                                b_t[:, k:k + 1])
                if dst8 is not None:
                    nc.scalar.copy(dst8[:, k], dst16[:, k])

        def qk_proj(wdram, bias_t, inv_t, srcs8, qT, kT, ecs):
            """fp8 DoubleRow projections. ec 0..7 -> qT (srcs8[0]);
            8..15 -> kT (srcs8[1])."""
            for ec in ecs:
                wt = wp.tile([128, KD, 128], fp8, tag="w8", name="wqkt")
                nc.sync.dma_start(wt[:], wdram[ec])
                src = srcs8[0] if ec < 8 else srcs8[1]
                dst = qT if ec < 8 else kT
                ps = mmp.tile([128, TM], f32, tag="mm", name="qkps")
                if proj_dr:
                    for j in range(KD // 2):
                        p = j
                        nc.tensor.matmul(ps[:], wt[:, 2 * p:2 * p + 2, :],
                                         src[:, 2 * p:2 * p + 2, :],
                                         start=(j == 0),
                                         stop=(j == KD // 2 - 1),
                                         perf_mode=DR)
                else:
                    for j in range(KD):
                        p = (j + ec + 1) % KD
                        nc.tensor.matmul(ps[:], wt[:, p], src[:, p],
                                         start=(j == 0), stop=(j == KD - 1))
                nc.scalar.activation(dst[:, ec % 8], ps[:], AF.Identity,
                                     bias=bias_t[:, ec:ec + 1], scale=inv_t)

        def v_proj_piece(wvt, inv_t, src8, vdst, half, tch):
            """One (half, tch) fp8 DR group of a v projection."""
            ps = mmp.tile([128, 512], f32, tag="mm", name="vps")
            if proj_dr:
                for j in range(KD // 2):
                    p = j
                    nc.tensor.matmul(
                        ps[:],
                        src8[:, 2 * p:2 * p + 2, 128 * tch:128 * tch + 128],
                        wvt[:, 2 * p:2 * p + 2, :],
                        start=(j == 0), stop=(j == KD // 2 - 1), perf_mode=DR)
            else:
                for j in range(KD):
                    p = (j + tch) % KD
                    nc.tensor.matmul(
                        ps[:], src8[:, p, 128 * tch:128 * tch + 128],
                        wvt[:, p], start=(j == 0), stop=(j == KD - 1))
            dst = vdst[:, tch, 520 * half:520 * half + 520]
            dst = dst.rearrange("p (h f) -> p h f", f=65)[:, :, 0:64]
            nc.scalar.activation(
                dst, ps[:].rearrange("p (h f) -> p h f", f=64),
                AF.Identity, scale=inv_t)

        def v_ones(vdst):
            nc.vector.memset(vdst[:, :, 64::65], 1.0)

        def attention(qT, kT, v8, onT8, banded, filler=None):
            """Scores fp16, exp->fp8 (x eb for banded), attnV fp8 DR."""
            for hp in range(H // 2):
                es = {0: [], 1: []}
                for par in (0, 1):
                    q = qT[64 * par:64 * par + 64, hp, :]
                    for cp in (0, 1):
                        sc = scorep.tile([128, 2, TM], f32, tag="sc",
                                         name="sc")
                        for i in (0, 1):
                            c = 2 * cp + i
                            k = kT[64 * par:64 * par + 64, hp,
                                   128 * c:128 * c + 128]
                            if banded:
                                u = U0[c]
                                nc.tensor.matmul(sc[:, i, 0:UW], k,
                                                 q[:, u:u + UW],
                                                 start=True, stop=True)
                            else:
                                nc.tensor.matmul(sc[:, i, :], k, q,
                                                 start=True, stop=True)
                        e = expp.tile([128, 2, TM], fp8, tag="e", name="e")
                        if banded:
                            nc.scalar.activation(e[:, :, 0:UW],
                                                 sc[:, :, 0:UW], AF.Exp)
                            nc.vector.tensor_tensor(
                                e[:, :, 0:UW], e[:, :, 0:UW],
                                ebt[:, 2 * cp:2 * cp + 2, :], OP.mult)
                        else:
                            nc.scalar.activation(e[:], sc[:], AF.Exp)
                        es[par].append(e)
                for par in (0, 1):
                    h = 2 * hp + par
                    op_ = mmp.tile([65, TM], f32, tag="mm", name="op")
                    if attnv_dr:
                        for cp in (0, 1):
                            stat = v8[:, 2 * cp:2 * cp + 2,
                                      65 * h:65 * h + 65]
                            e = es[par][cp]
                            if banded:
                                u = U0[2 * cp]
                                nc.tensor.matmul(op_[:, u:u + UW], stat,
                                                 e[:, :, 0:UW],
                                                 start=(cp == 0),
                                                 stop=(cp == 1), perf_mode=DR)
                            else:
                                nc.tensor.matmul(op_[:], stat, e[:],
                                                 start=(cp == 0),
                                                 stop=(cp == 1), perf_mode=DR)
                    else:
                        for c in range(4):
                            stat = v8[:, c, 65 * h:65 * h + 65]
                            e = es[par][c // 2]
                            if banded:
                                u = U0[c]
                                nc.tensor.matmul(op_[:, u:u + UW], stat,
                                                 e[:, c % 2, 0:UW],
                                                 start=(c == 0), stop=(c == 3))
                            else:
                                nc.tensor.matmul(op_[:], stat, e[:, c % 2, :],
                                                 start=(c == 0), stop=(c == 3))
                    rr = smrp.tile([1, TM], f32r, tag="smr", name="rr")
                    with nc.allow_low_precision(reason="softmax denom recip"):
                        nc.vector.reciprocal(rr[:], op_[64:65, :])
                    bc = mmp.tile([64, TM], f32, tag="mm", name="bct")
                    nc.tensor.matmul(bc[:], ones_r[0:1, 0:64], rr[:],
                                     start=True, stop=True)
                    bcs = bcp.tile([64, TM], f16, tag="bcs", name="bcs")
                    nc.scalar.copy(bcs[:], bc[:])
                    if par == 0:
                        nc.vector.tensor_tensor(onT8[0:64, hp, :],
                                                op_[0:64, :], bcs[:], OP.mult)
                    else:
                        sc8 = scrp.tile([64, TM], fp8, tag="shift", name="sc8")
                        nc.vector.tensor_tensor(sc8[:], op_[0:64, :], bcs[:],
                                                OP.mult)
                        nc.sync.dma_start(onT8[64:128, hp, :], sc8[:])
                if filler is not None:
                    filler(hp)

        def out_proj_res_ln(wdram, bias_t, inv_t, onT8, resT, g_t, bt_t,
                            dst16, dst8):
            xres = t16("xres")
            for ec in range(8):
                wt = wp.tile([128, KD, 128], fp8, tag="w8", name="wot")
                nc.sync.dma_start(wt[:], wdram[ec])
                ps = mmp.tile([128, TM], f32, tag="mm", name="ops")
                for j in range(KD // 2):
                    p = j
                    nc.tensor.matmul(ps[:], wt[:, 2 * p:2 * p + 2, :],
                                     onT8[:, 2 * p:2 * p + 2, :],
                                     start=(j == 0), stop=(j == KD // 2 - 1),
                                     perf_mode=DR)
                if bz:
                    nc.vector.scalar_tensor_tensor(
                        xres[:, ec], ps[:], inv_t, resT[:, ec],
                        OP.mult, OP.add)
                else:
                    tb = tmpp.tile([128, TM], f16, tag="t1", name="tb")
                    nc.vector.scalar_tensor_tensor(
                        tb[:], ps[:], inv_t, bias_t[:, ec:ec + 1],
                        OP.mult, OP.add)
                    nc.vector.tensor_tensor(xres[:, ec], tb[:], resT[:, ec],
                                            OP.add)
            ln(xres, g_t, bt_t, dst16, dst8)

        # --- main loop over the core's batch elems -------------------------
        pending = {}   # b -> (x8, v8) prepared by previous b's CA filler
        for b in range(BPC):
            if b in pending:
                x8, v8 = pending.pop(b)
            else:
                x8 = t8("x8")
                nc.sync.dma_start(
                    x8[:], xin8[b].rearrange("k p t -> p k t"))
                v8 = None
            xT = t16("xT")
            for hh in range(2):
                nc.sync.dma_start(
                    xT[:, 4 * hh:4 * hh + 4, :],
                    xin16[b, 4 * hh:4 * hh + 4].rearrange("k p t -> p k t"))

            # A: SA q/k projections (k first: scores stationary)
            qT = t16("qT")
            kT = t16("kT")
            qk_proj(wqk_sa, bqk_sa_t, inv_qk_sa, (x8, x8), qT, kT,
                    list(range(8, 16)) + list(range(8)))
            # SA v projection (b=0 only; later b's were prefetched by the
            # previous b's CA-attention filler)
            if v8 is None:
                v8 = vp.tile([128, TCH, H * 65], fp8, tag="v8", name="v8")
                v_ones(v8)
                for half in range(2):
                    wvt = wvp.tile([128, KD, 512], fp8, tag="wv", name="wvt")
                    nc.sync.dma_start(wvt[:], wv_sa[half])
                    for tch in range(TCH):
                        v_proj_piece(wvt, inv_v_sa, x8, v8, half, tch)

            # B: audio fp8 load + CA k/v interleaved into SA attention
            a8 = t8("a8")
            nc.sync.dma_start(a8[:], ain8[b].rearrange("k p t -> p k t"))
            qT2 = t16("qT2")
            kT2 = t16("kT2")
            v28 = vp.tile([128, TCH, H * 65], fp8, tag="v8", name="v28")
            v_ones(v28)
            wvt2_holder = {}

            def sa_filler(hp):
                if hp is None:
                    return
                # one CA-k ec group + one CA-v piece per SA hp
                ec = 8 + hp
                wt = wp.tile([128, KD, 128], fp8, tag="w8", name="wqkt2")
                nc.sync.dma_start(wt[:], wqk_ca[ec])
                ps = mmp.tile([128, TM], f32, tag="mm", name="qkps2")
                for j in range(KD // 2):
                    p = j
                    nc.tensor.matmul(ps[:], wt[:, 2 * p:2 * p + 2, :],
                                     a8[:, 2 * p:2 * p + 2, :],
                                     start=(j == 0), stop=(j == KD // 2 - 1),
                                     perf_mode=DR)
                nc.scalar.activation(kT2[:, hp], ps[:], AF.Identity,
                                     bias=bqk_ca_t[:, ec:ec + 1],
                                     scale=inv_qk_ca)
                half, tch = hp // 4, hp % 4
                if tch == 0:
                    wvt2 = wvp.tile([128, KD, 512], fp8, tag="wv",
                                    name="wvt2")
                    nc.sync.dma_start(wvt2[:], wv_ca[half])
                    wvt2_holder[half] = wvt2
                v_proj_piece(wvt2_holder[half], inv_v_ca, a8, v28, half, tch)

            onT8 = t8("onT8")
            attention(qT, kT, v8, onT8, banded=False,
                      filler=None if nofill else sa_filler)
            if nofill:
                for hp in range(8):
                    sa_filler(hp)

            # C: SA out-proj + residual + LN1
            x1T = t16("x1T")
            x1T8 = t8("x1T8")
            out_proj_res_ln(wo_sa, bo_sa_t, inv_o_sa, onT8, xT,
                            n1g_t, n1b_t, x1T, x1T8)

            # D: CA q projection from x1
            qk_proj(wqk_ca, bqk_ca_t, inv_qk_ca, (x1T8, None), qT2, kT2,
                    list(range(8)))

            # E: CA attention (banded), next-b v_proj interleaved
            onT28 = t8("onT28")
            if b + 1 < BPC:
                nx8 = t8("x8")
                nc.sync.dma_start(
                    nx8[:], xin8[b + 1].rearrange("k p t -> p k t"))
                nv8 = vp.tile([128, TCH, H * 65], fp8, tag="v8", name="nv8")
                v_ones(nv8)
                pending[b + 1] = (nx8, nv8)
                nwv_holder = {}

                def ca_filler(hp):
                    half, tch = hp // 4, hp % 4
                    if tch == 0:
                        nwvt = wvp.tile([128, KD, 512], fp8, tag="wv",
                                        name="nwvt")
                        nc.sync.dma_start(nwvt[:], wv_sa[half])
                        nwv_holder[half] = nwvt
                    v_proj_piece(nwv_holder[half], inv_v_sa, nx8, nv8,
                                 half, tch)
            else:
                ca_filler = None
            attention(qT2, kT2, v28, onT28, banded=True,
                      filler=None if nofill else ca_filler)
            if nofill and ca_filler is not None:
                for hp in range(8):
                    ca_filler(hp)

            # F: CA out-proj (tanh(gate) folded on host) + residual + LNc
            x2T = t16("x2T")
            x2T8 = t8("x2T8") if lin1_fp8 else None
            out_proj_res_ln(wo_ca, bo_ca_t, inv_o_ca, onT28, x1T,
                            ncg_t, ncb_t, x2T, x2T8)

            # G: lin1 + gelu -> hq
            hq = hqp.tile([128, KF, TM], fp8 if lin2_fp8 else f16,
                          tag="hq", name="hq")
            for fc in range(KF):
                wpool = wp if lin1_fp8 else w16p
                wt = wpool.tile([128, KD, 128], fp8 if lin1_fp8 else f16,
                                tag="w8" if lin1_fp8 else "w16", name="w1t")
                nc.sync.dma_start(wt[:], w1[fc])
                ps = mmp.tile([128, TM], f32, tag="mm", name="hps")
                if lin1_fp8:
                    for j in range(KD // 2):
                        p = j
                        nc.tensor.matmul(ps[:], wt[:, 2 * p:2 * p + 2, :],
                                         x2T8[:, 2 * p:2 * p + 2, :],
                                         start=(j == 0),
                                         stop=(j == KD // 2 - 1),
                                         perf_mode=DR)
                else:
                    for j in range(KD):
                        k = (j + fc) % KD
                        nc.tensor.matmul(ps[:], wt[:, k], x2T[:, k],
                                         start=(j == 0), stop=(j == KD - 1))
                nc.scalar.activation(hq[:, fc], ps[:], AF.Gelu,
                                     bias=b1_t[:, fc:fc + 1], scale=inv_1)

            # H: lin2 + residual + LN2 -> out
            xres2 = t16("xres2")
            for ec in range(8):
                wt = w2p.tile([128, KF, 128], fp8 if lin2_fp8 else f16,
                              tag="w2", name="w2t")
                nc.sync.dma_start(wt[:], w2[ec])
                ps = mmp.tile([128, TM], f32, tag="mm", name="fps")
                if lin2_fp8:
                    for j in range(KF // 2):
                        p = j
                        nc.tensor.matmul(ps[:], wt[:, 2 * p:2 * p + 2, :],
                                         hq[:, 2 * p:2 * p + 2, :],
                                         start=(j == 0),
                                         stop=(j == KF // 2 - 1),
                                         perf_mode=DR)
                else:
                    for j in range(KF):
                        p = (j + 4 * ec) % KF
                        nc.tensor.matmul(ps[:], wt[:, p], hq[:, p],
                                         start=(j == 0), stop=(j == KF - 1))
                if bz:
                    nc.vector.scalar_tensor_tensor(
                        xres2[:, ec], ps[:], inv_2, x2T[:, ec],
                        OP.mult, OP.add)
                else:
                    tb = tmpp.tile([128, TM], f16, tag="t1", name="tb2")
                    nc.vector.scalar_tensor_tensor(
                        tb[:], ps[:], inv_2, b2_t[:, ec:ec + 1],
                        OP.mult, OP.add)
                    nc.vector.tensor_tensor(xres2[:, ec], tb[:], x2T[:, ec],
                                            OP.add)
            outT = t16("outT")
            ln(xres2, n2g_t, n2b_t, outT, None)
            for hh in range(2):
                nc.sync.dma_start(
                    out[b, 4 * hh:4 * hh + 4].rearrange("k p t -> p k t"),
                    outT[:, 4 * hh:4 * hh + 4, :])

    return nc


def _get_module(cfg=None):
    if cfg is None:
        cfg = _CACHE.get("last_cfg",
                         (True, True, False, False, True, False, True))
    _CACHE["last_cfg"] = cfg
    key = ("nc", cfg)
    if key not in _CACHE:
        _install_patches()
        _CACHE[key] = _build_module(*cfg)
    return _CACHE[key]


# ---------------------------------------------------------------------------
# Host-side prep + execution
# ---------------------------------------------------------------------------
def _beat_bias(beats):
    beats = np.asarray(beats).astype(np.int64).ravel()
    bias = np.zeros(TA, np.float32)
    l_idx = np.where(beats > 0, beats - 1, 0)
    l_val = np.where(beats > 0, BW * 0.5, 0.0).astype(np.float32)
    r_idx = np.where(beats < TA - 1, beats + 1, TA - 1)
    r_val = np.where(beats < TA - 1, BW * 0.5, 0.0).astype(np.float32)
    np.maximum.at(bias, l_idx, l_val)
    np.maximum.at(bias, r_idx, r_val)
    np.maximum.at(bias, beats, np.float32(BW))
    return bias


def _temporal_bias():
    scale = (TA - 1) / (TM - 1)
    audio_pos = np.arange(TM, dtype=np.float32) * scale
    diff = audio_pos[:, None] - np.arange(TA, dtype=np.float32)[None, :]
    return (-(diff ** 2) / (2.0 * SIGMA ** 2)).astype(np.float32)


def _chunk_w(w, n_out_chunks, n_in_chunks, dt):
    # w: [E, Dk] row-major -> [ec, p(in), kc, j(out)]
    E, Dk = w.shape
    return np.ascontiguousarray(
        w.reshape(n_out_chunks, E // n_out_chunks, n_in_chunks, Dk // n_in_chunks)
        .transpose(0, 3, 2, 1).astype(dt))


def _pp(vec):
    # [n*128] -> [128, n] per-partition layout
    v = np.asarray(vec, np.float32).reshape(-1, 128)
    return np.ascontiguousarray(v.T)


def _pow2scale(w, target=192.0):
    m = float(np.abs(w).max())
    if m == 0:
        return 1.0
    return float(2.0 ** np.floor(np.log2(target / m)))


def kernel(**inputs):
    import ml_dtypes
    from concourse.bass_utils import run_bass_kernel_spmd

    fp8 = ml_dtypes.float8_e4m3
    f16 = np.float16
    f32 = np.float32

    src = np.asarray(inputs["src"], f32)
    audio = np.asarray(inputs["audio_memory"], f32)
    beats = inputs["beat_frames"]

    ln_id = not (
        np.any(np.asarray(inputs["n1_b"])) or np.any(np.asarray(inputs["nc_b"]))
        or np.any(np.asarray(inputs["n2_b"]))
        or np.any(np.asarray(inputs["n1_g"]) != 1.0)
        or np.any(np.asarray(inputs["nc_g"]) != 1.0)
        or np.any(np.asarray(inputs["n2_g"]) != 1.0))

    sa_in_w = np.asarray(inputs["sa_in_w"], f32)
    sa_in_b = np.asarray(inputs["sa_in_b"], f32)
    sa_out_w = np.asarray(inputs["sa_out_w"], f32)
    sa_out_b = np.asarray(inputs["sa_out_b"], f32)
    ca_in_w = np.asarray(inputs["ca_in_w"], f32)
    ca_in_b = np.asarray(inputs["ca_in_b"], f32)
    ca_out_w = np.asarray(inputs["ca_out_w"], f32)
    ca_out_b = np.asarray(inputs["ca_out_b"], f32)
    gate = float(np.asarray(inputs["gate"]))
    tg = float(np.tanh(gate))
    lin1_b = np.asarray(inputs["lin1_b"], f32)
    lin2_b = np.asarray(inputs["lin2_b"], f32)

    # SA: fold 1/8 score scale into q weights+bias; v-bias into out-proj bias.
    wqk_sa_eff = np.concatenate([sa_in_w[:D] / 8.0, sa_in_w[D:2 * D]], axis=0)
    bqk_sa_eff = np.concatenate([sa_in_b[:D] / 8.0, sa_in_b[D:2 * D]])
    bo_sa_eff = sa_out_b + sa_out_w @ sa_in_b[2 * D:]
    wqk_ca_eff = np.concatenate([ca_in_w[:D] / 8.0, ca_in_w[D:2 * D]], axis=0)
    bqk_ca_eff = np.concatenate([ca_in_b[:D] / 8.0, ca_in_b[D:2 * D]])
    wo_ca_eff = tg * ca_out_w
    bo_ca_eff = tg * (ca_out_b + ca_out_w @ ca_in_b[2 * D:])

    bz = not (np.any(bqk_sa_eff) or np.any(bo_sa_eff) or np.any(bqk_ca_eff)
              or np.any(bo_ca_eff) or np.any(lin2_b))
    lin1_fp8 = os.environ.get("K_LIN1_FP8", "0") == "1"
    lin2_fp8 = os.environ.get("K_LIN2_FP8", "0") == "1"
    attnv_dr = os.environ.get("K_ATTNV_DR", "1") == "1"
    proj_dr = os.environ.get("K_PROJ_DR", "0") == "1"
    nofill = os.environ.get("K_NOFILL", "1") == "1"
    nc = _get_module((ln_id, bz, lin1_fp8, lin2_fp8, attnv_dr, proj_dr,
                      nofill))

    # feature-major activations: [B, KD, 128, T]
    xfm = src.transpose(1, 2, 0).reshape(B, KD, 128, TM)
    afm = audio.transpose(1, 2, 0).reshape(B, KD, 128, TA)
    xin16_all = np.ascontiguousarray(xfm.astype(f16))
    xin8_all = np.ascontiguousarray(xfm.astype(fp8))
    ain8_all = np.ascontiguousarray(afm.astype(fp8))

    # fp8 weight scaling (pow2 per tensor)
    s_qk_sa = _pow2scale(wqk_sa_eff)
    s_v_sa = _pow2scale(sa_in_w[2 * D:])
    s_o_sa = _pow2scale(sa_out_w)
    s_qk_ca = _pow2scale(wqk_ca_eff)
    s_v_ca = _pow2scale(ca_in_w[2 * D:])
    s_o_ca = _pow2scale(wo_ca_eff)
    w1_f = np.asarray(inputs["lin1_w"], f32)
    w2_f = np.asarray(inputs["lin2_w"], f32)
    s_1 = _pow2scale(w1_f) if lin1_fp8 else 1.0
    s_2 = _pow2scale(w2_f) if lin2_fp8 else 1.0

    # exp(bias) tiles: [tk-part 128, chunk 4, tq-local 384], 0 outside band
    bias = _temporal_bias() + _beat_bias(beats)[None, :]  # [tq, tk]
    bT = bias.T  # [tk, tq]
    ebt = np.zeros((128, 4, UW), f32)
    for c in range(4):
        lo, w = TQR[c]
        u = U0[c]
        blk = np.exp(bT[128 * c:128 * c + 128, lo:lo + w])
        ebt[:, c, lo - u:lo - u + w] = blk
    ebt = ebt.astype(f16)

    # v half-major weight layout: [half][in-part 128, kd, 512 out]
    def _wv_prep(wv, s):
        a = _chunk_w(wv * s, 2, KD, fp8)  # [2, 128, KD, 512]
        return np.ascontiguousarray(a)

    weights = {
        "wqk_sa": _chunk_w(wqk_sa_eff * s_qk_sa, 16, KD, fp8),
        "wv_sa": _wv_prep(sa_in_w[2 * D:], s_v_sa),
        "wo_sa": _chunk_w(sa_out_w * s_o_sa, 8, KD, fp8),
        "wqk_ca": _chunk_w(wqk_ca_eff * s_qk_ca, 16, KD, fp8),
        "wv_ca": _wv_prep(ca_in_w[2 * D:], s_v_ca),
        "wo_ca": _chunk_w(wo_ca_eff * s_o_ca, 8, KD, fp8),
        "w1": _chunk_w(w1_f * s_1, KF, KD, fp8 if lin1_fp8 else f16),
        "w2": _chunk_w(w2_f * s_2, 8, KF, fp8 if lin2_fp8 else f16),
        "ppb": np.ascontiguousarray(np.concatenate([
            _pp(bqk_sa_eff),
            _pp(bo_sa_eff), _pp(bqk_ca_eff),
            _pp(bo_ca_eff),
            _pp(lin1_b),
            _pp(lin2_b),
            _pp(np.asarray(inputs["n1_g"], f32)),
            _pp(np.asarray(inputs["n1_b"], f32)),
            _pp(np.asarray(inputs["nc_g"], f32)),
            _pp(np.asarray(inputs["nc_b"], f32)),
            _pp(np.asarray(inputs["n2_g"], f32)),
            _pp(np.asarray(inputs["n2_b"], f32)),
            np.full((128, 1), 1.0 / s_qk_sa, f32),
            np.full((128, 1), 1.0 / s_v_sa, f32),
            np.full((128, 1), 1.0 / s_o_sa, f32),
            np.full((128, 1), 1.0 / s_qk_ca, f32),
            np.full((128, 1), 1.0 / s_v_ca, f32),
            np.full((128, 1), 1.0 / s_o_ca, f32),
            np.full((128, 1), 1.0 / s_1, f32),
            np.full((128, 1), 1.0 / s_2, f32),
        ], axis=1)),
        "eb": ebt,
        "onescol": np.ones((128, 1), f32),
        "onesrow": np.ones((1, 128), f32),
    }

    in_maps = []
    for c in range(NCORES):
        m = dict(weights)
        m["xin16"] = np.ascontiguousarray(xin16_all[BPC * c:BPC * (c + 1)])
        m["xin8"] = np.ascontiguousarray(xin8_all[BPC * c:BPC * (c + 1)])
        m["ain8"] = np.ascontiguousarray(ain8_all[BPC * c:BPC * (c + 1)])
        in_maps.append(m)

    res = run_bass_kernel_spmd(nc, in_maps, core_ids=list(range(NCORES)))
    outs = [r["out"] for r in res.results]  # each [BPC, KD, 128, TM] f16
    full = np.concatenate(outs, axis=0).astype(np.float32)
    return np.ascontiguousarray(
        full.reshape(B, D, TM).transpose(2, 0, 1))


# revision 19
# speedup vs baseline: 1.7331x; 1.7331x over previous
"""AudioCondTransformerEncoderLayer on 8 Trainium2 NeuronCores.

v3 strategy (TM=TA=512, B=32, D=1024, H=16, DFF=4096, 4 batch elems/core):
  - Data-parallel over batch across 8 cores; per-core per-b pipeline with
    cross-b overlap via pooled buffers.
  - fp16 (not bf16) residual stream / q/k / LN / exp-bias tiles: same
    PE/DVE/ACT cost as bf16, 4x less rounding noise (rel err 5.7e-3 vs
    1.3e-2 for the bf16 version).
  - fp8(e4m3) operands for all projections (q/k/v/out, both attentions):
    plain (non-DoubleRow) matmuls — same PE speed as bf16, but halves
    weight DMA and SBUF, and enables the DoubleRow attnV. DoubleRow for
    the projections measured 2-2.5us/instr in-kernel (fragile to any PE
    stream disruption) and is disabled by default (K_PROJ_DR=1 to try).
    Weights pre-scaled by pow2 per-tensor factors on host; descale via
    ACT drain `scale=` or folded into stt residual drains (inv-scale
    columns live in ppb).
  - attnV uses fp8 DoubleRow ([128,2,65] stationary v pairs, fp8 exp
    moving): measured neutral-to-positive vs non-DR (K_ATTNV_DR=0 to
    disable).
  - Cross-attention bias: host-precomputed exp(bias) fp16 tiles
    multiplied into the fp8 exp tiles on DVE (frees the PE
    identity-matmul bias adds of v2). Banded: tk-chunk pairs (0,1)/(2,3)
    share 384-wide tq union windows; eb is zero-padded outside each
    chunk's true band so DoubleRow attnV over the union window is exact.
  - Softmax denominator via ones column in v (65-col attnV stationary);
    reciprocal on DVE, broadcast across partitions via PE outer product.
  - LayerNorm: partition sums via ones-column fp16 matmuls, variance via
    fused stt, mean/invstd broadcast via PE outer products; apply on DVE
    writes both fp16 (residual) and fp8 (next-GEMM input) copies.
  - "Filler" interleaving of CA k/v projections into the SA attention hp
    loop exists (K_NOFILL=0) but measured neutral-to-negative on HW;
    disabled by default.
  - PSUM: scores 2x2-bank + mm 3x1 + LN-sums 1 = 8 banks.
  - NOTE: HW timing on this axon stack is bimodal per process (~2.1ms or
    ~4.0ms for identical NEFFs, likely a PE DVFS/p-state lock); compare
    configs by min over 3+ separate runs.
"""

import os

import numpy as np

# ---------------------------------------------------------------------------
# Problem constants
# ---------------------------------------------------------------------------
D = 1024
H = 16
HD = 64
TM = 512
TA = 512
B = 32
DFF = 4096
NCORES = 8
BPC = B // NCORES          # batch elems per core
SIGMA = 4.0
BW = 2.0
LN_EPS = 1e-5
KD = D // 128              # 8 d-chunks
KF = DFF // 128            # 32 ff-chunks
TCH = TM // 128            # 4 token chunks

# banded CA: true tq band per tk-chunk, within 384-wide union windows
TQR = [(0, 256), (0, 384), (128, 384), (256, 256)]
U0 = [0, 0, 128, 128]      # union window start per chunk
UW = 384                   # union window width

_CACHE = {}


# ---------------------------------------------------------------------------
# Walrus workaround: this container's walrus build rejects >1 sync-wait per
# instruction. Split excess waits onto preceding same-engine NOPs, and move
# the tail drain's waits onto SP NOPs.
# ---------------------------------------------------------------------------
def _install_patches():
    if _CACHE.get("patched"):
        return
    import concourse.mybir as mybir
    import concourse.tile as tile
    import concourse.tile_utils as tile_utils
    from concourse.vector_clock import ScopedClock

    tile_utils.max_sbuf_usage = 208 * 1024

    _orig_commit = tile.TileContext._commit_instruction

    def _split_commit(self, inst, lazy_reg_writes=True):
        si = inst.sync_info
        if (
            si is not None
            and len(si.on_wait) > 1
            and inst.engine != mybir.EngineType.Unassigned
        ):
            waits = list(si.on_wait)
            inst.sync_info = mybir.SyncInfo(
                on_wait=waits[:1], on_update=list(si.on_update)
            )
            for w in waits[1:]:
                nop = mybir.InstNoOp(
                    name=self.nc.get_next_instruction_name(),
                    ins=[],
                    outs=[],
                    engine=inst.engine,
                    sync_info=mybir.SyncInfo(on_wait=[w], on_update=[]),
                )
                nop.debug = inst.debug
                _orig_commit(self, nop, lazy_reg_writes=False)
        return _orig_commit(self, inst, lazy_reg_writes=lazy_reg_writes)

    tile.TileContext._commit_instruction = _split_commit

    def _patched_drain_and_barrier(self, tick_clock, wait_clock):
        carrier = self.nc.sync.nop(nofuse=True)
        wait_clock.add_sem_waits(
            carrier.ins, ScopedClock({None: tick_clock.global_clock})
        )
        si = carrier.ins.sync_info
        if si is not None and len(si.on_wait) > 1:
            waits = list(si.on_wait)
            carrier.ins.sync_info = mybir.SyncInfo(
                on_wait=waits[:1], on_update=list(si.on_update)
            )
            for w in waits[1:]:
                extra = self.nc.sync.nop(nofuse=True)
                extra.ins.sync_info = mybir.SyncInfo(on_wait=[w], on_update=[])
        self.nc.sync.drain()
        self.nc.all_engine_barrier()
        popped = self.nc._tile_sem_poison_stack.pop()
        assert popped is self._sem_poison
        self.nc.clear_and_free_semaphores(list(self.sems.allocated().values()))
        self.nc.all_engine_barrier()

    tile.TileContext._drain_and_barrier = _patched_drain_and_barrier
    _CACHE["patched"] = True


# ---------------------------------------------------------------------------
# Device module
# ---------------------------------------------------------------------------
def _build_module(ln_id, bz, lin1_fp8, lin2_fp8, attnv_dr=True, proj_dr=True, nofill=False):
    """ln_id: LN affine is identity; bz: all GEMM biases are zero."""
    from contextlib import ExitStack

    import concourse.bass as bass
    import concourse.mybir as mybir
    import concourse.tile as tile

    f32 = mybir.dt.float32
    f32r = mybir.dt.float32r
    f16 = mybir.dt.float16
    fp8 = mybir.dt.float8e4
    AF = mybir.ActivationFunctionType
    OP = mybir.AluOpType
    DR = mybir.MatmulPerfMode.DoubleRow

    nc = bass.Bass()

    def din(name, shape, dt):
        return nc.dram_tensor(name, shape, dt, kind="ExternalInput")

    xin16 = din("xin16", (BPC, KD, 128, TM), f16)
    xin8 = din("xin8", (BPC, KD, 128, TM), fp8)
    ain8 = din("ain8", (BPC, KD, 128, TA), fp8)
    wqk_sa = din("wqk_sa", (16, 128, KD, 128), fp8)
    wv_sa = din("wv_sa", (2, 128, KD, 512), fp8)
    wo_sa = din("wo_sa", (8, 128, KD, 128), fp8)
    wqk_ca = din("wqk_ca", (16, 128, KD, 128), fp8)
    wv_ca = din("wv_ca", (2, 128, KD, 512), fp8)
    wo_ca = din("wo_ca", (8, 128, KD, 128), fp8)
    w1 = din("w1", (KF, 128, KD, 128), fp8 if lin1_fp8 else f16)
    w2 = din("w2", (8, 128, KF, 128), fp8 if lin2_fp8 else f16)
    # per-partition bias/gain/inv-scale columns:
    # [bqk_sa(16) bo_sa(8) bqk_ca(16) bo_ca(8) b1(32) b2(8)
    #  n1g n1b ncg ncb n2g n2b (6x8)
    #  inv: qk_sa v_sa o_sa qk_ca v_ca o_ca lin1 lin2 (8x1)] = 144 cols
    ppb = din("ppb", (128, 144), f32)
    eb = din("eb", (128, 4, UW), f16)   # exp(bias) per tk-chunk, 0-padded
    onescol = din("onescol", (128, 1), f32)
    onesrow = din("onesrow", (1, 128), f32)

    out = nc.dram_tensor("out", (BPC, KD, 128, TM), f16, kind="ExternalOutput")

    with tile.TileContext(nc) as tc, ExitStack() as ctx:
        cpool = ctx.enter_context(tc.tile_pool(name="consts", bufs=1))
        # fp16 stream tensors [128, KD, 512] = 8KB/part
        a16 = ctx.enter_context(tc.tile_pool(name="a16", bufs=7))
        # fp8 stream tensors [128, KD, 512] = 4KB/part
        a8p = ctx.enter_context(tc.tile_pool(name="a8p", bufs=6))
        vp = ctx.enter_context(tc.tile_pool(name="vp", bufs=3))       # v fp8
        hqp = ctx.enter_context(
            tc.tile_pool(name="hqp", bufs=2 if lin2_fp8 else 1))      # h
        expp = ctx.enter_context(tc.tile_pool(name="expp", bufs=8))   # e fp8
        wp = ctx.enter_context(tc.tile_pool(name="wqkp", bufs=8))     # 1KB w
        w16p = ctx.enter_context(tc.tile_pool(name="w16p", bufs=4))   # 2KB w1
        wvp = ctx.enter_context(tc.tile_pool(name="wvp", bufs=2))     # 4KB wv
        w2p = ctx.enter_context(tc.tile_pool(name="w2p", bufs=2))     # 4KB w2
        mip = ctx.enter_context(tc.tile_pool(name="mip", bufs=2))
        smp = ctx.enter_context(tc.tile_pool(name="small", bufs=2))
        smrp = ctx.enter_context(tc.tile_pool(name="smallr", bufs=2))
        mbp = ctx.enter_context(tc.tile_pool(name="mbp", bufs=2))     # miB
        bcp = ctx.enter_context(tc.tile_pool(name="bcast", bufs=3))   # bcs
        tmpp = ctx.enter_context(tc.tile_pool(name="tmp", bufs=3))
        sqp = ctx.enter_context(tc.tile_pool(name="sq", bufs=4))
        scrp = ctx.enter_context(tc.tile_pool(name="scratch", bufs=2))
        # psum: scorep 2x2-bank + mmp 3 + pss 1 = 8 banks
        scorep = ctx.enter_context(
            tc.tile_pool(name="scorep", bufs=2, space="PSUM"))
        mmp = ctx.enter_context(tc.tile_pool(name="mmp", bufs=3, space="PSUM"))
        pss = ctx.enter_context(tc.tile_pool(name="pss", bufs=1, space="PSUM"))

        # --- constants -----------------------------------------------------
        ones_c16 = cpool.tile([128, 1], f16, name="ones_c16")
        ones_c32 = cpool.tile([128, 1], f32, name="ones_c32")
        nc.sync.dma_start(ones_c32[:], onescol[:, :])
        nc.vector.tensor_copy(ones_c16[:], ones_c32[:])
        ones_r = cpool.tile([1, 128], f32r, name="ones_r")
        nc.sync.dma_start(ones_r[:], onesrow[:, :].bitcast(f32r))
        ebt = cpool.tile([128, 4, UW], f16, name="ebt")
        nc.sync.dma_start(ebt[:], eb[:, :, :])
        eps_t = cpool.tile([1, 1], f32, name="eps_t")
        nc.vector.memset(eps_t[:], LN_EPS)

        ppb_t = cpool.tile([128, 144], f32, name="ppb_t")
        nc.sync.dma_start(ppb_t[:], ppb[:, :])
        _off = [0]

        def pp_view(n):
            o = _off[0]
            _off[0] += n
            return ppb_t[:, o:o + n]

        bqk_sa_t = pp_view(16)
        bo_sa_t = pp_view(8)
        bqk_ca_t = pp_view(16)
        bo_ca_t = pp_view(8)
        b1_t = pp_view(KF)
        b2_t = pp_view(8)
        n1g_t, n1b_t = pp_view(8), pp_view(8)
        ncg_t, ncb_t = pp_view(8), pp_view(8)
        n2g_t, n2b_t = pp_view(8), pp_view(8)
        inv_qk_sa = pp_view(1)
        inv_v_sa = pp_view(1)
        inv_o_sa = pp_view(1)
        inv_qk_ca = pp_view(1)
        inv_v_ca = pp_view(1)
        inv_o_ca = pp_view(1)
        inv_1 = pp_view(1)
        inv_2 = pp_view(1)

        def t16(name):
            return a16.tile([128, KD, TM], f16, tag="a16", name=name)

        def t8(name):
            return a8p.tile([128, KD, TM], fp8, tag="a8", name=name)

        # --- helpers -------------------------------------------------------
        def ln(y, g_t, b_t, dst16, dst8):
            """LayerNorm over the feature axis of y [128,KD,T] fp16."""
            ps_s = pss.tile([1, TM], f32, tag="sps", name="ps_s")
            for k in range(KD):
                nc.tensor.matmul(ps_s[:], ones_c16[:], y[:, k],
                                 start=(k == 0), stop=(k == KD - 1))
            mi = mip.tile([1, 2, TM], f32r, tag="mi", name="mi")
            with nc.allow_low_precision(reason="ln mean f32r for bcast mm"):
                nc.scalar.mul(mi[:, 0], ps_s[:], 1.0 / D)
            miB = mbp.tile([128, 2, TM], f16, tag="bcl", name="miB")
            bmu = mmp.tile([128, TM], f32, tag="mm", name="bmu")
            nc.tensor.matmul(bmu[:], ones_r[:], mi[:, 0], start=True, stop=True)
            nc.scalar.copy(miB[:, 0], bmu[:])
            ps_q = pss.tile([1, TM], f32, tag="sps", name="ps_q")
            for k in range(KD):
                sq = sqp.tile([128, TM], f16, tag="sq", name="sq")
                if k % 2 == 0:
                    nc.vector.tensor_tensor(sq[:], y[:, k], y[:, k], OP.mult)
                else:
                    nc.scalar.activation(sq[:], y[:, k], AF.Square)
                nc.tensor.matmul(ps_q[:], ones_c16[:], sq[:],
                                 start=(k == 0), stop=(k == KD - 1))
            m2 = smp.tile([1, TM], f32, tag="sm", name="m2")
            nc.vector.tensor_tensor(m2[:], mi[:, 0].bitcast(f32),
                                    mi[:, 0].bitcast(f32), OP.mult)
            var = smp.tile([1, TM], f32, tag="sm", name="var")
            nc.vector.scalar_tensor_tensor(var[:], ps_q[:], 1.0 / D, m2[:],
                                           OP.mult, OP.subtract)
            sd = smp.tile([1, TM], f32, tag="sm", name="sd")
            nc.scalar.activation(sd[:], var[:], AF.Sqrt, bias=eps_t[:])
            with nc.allow_low_precision(reason="ln invstd recip"):
                nc.vector.reciprocal(mi[:, 1], sd[:])
            biv = mmp.tile([128, TM], f32, tag="mm", name="biv")
            nc.tensor.matmul(biv[:], ones_r[:], mi[:, 1], start=True, stop=True)
            nc.scalar.copy(miB[:, 1], biv[:])
            for k in range(KD):
                t1 = tmpp.tile([128, TM], f16, tag="t1", name="t1")
                nc.vector.tensor_tensor(t1[:], y[:, k], miB[:, 0], OP.subtract)
                if ln_id:
                    nc.vector.tensor_tensor(dst16[:, k], t1[:], miB[:, 1],
                                            OP.mult)
                else:
                    t2 = tmpp.tile([128, TM], f16, tag="t1", name="t2")
                    nc.vector.scalar_tensor_tensor(
                        t2[:], t1[:], g_t[:, k:k + 1], miB[:, 1],
                        OP.mult, OP.mult)
                    nc.vector.tensor_scalar_add(dst16[:, k], t2[:],
                                                b_t[:, k:k + 1])
                if dst8 is not None:
                    nc.scalar.copy(dst8[:, k], dst16[:, k])

        def qk_proj(wdram, bias_t, inv_t, srcs8, qT, kT, ecs):
            """fp8 DoubleRow projections. ec 0..7 -> qT (srcs8[0]);
            8..15 -> kT (srcs8[1])."""
            for ec in ecs:
                wt = wp.tile([128, KD, 128], fp8, tag="w8", name="wqkt")
                nc.sync.dma_start(wt[:], wdram[ec])
                src = srcs8[0] if ec < 8 else srcs8[1]
                dst = qT if ec < 8 else kT
                ps = mmp.tile([128, TM], f32, tag="mm", name="qkps")
                if proj_dr:
                    for j in range(KD // 2):
                        p = j
                        nc.tensor.matmul(ps[:], wt[:, 2 * p:2 * p + 2, :],
                                         src[:, 2 * p:2 * p + 2, :],
                                         start=(j == 0),
                                         stop=(j == KD // 2 - 1),
                                         perf_mode=DR)
                else:
                    for j in range(KD):
                        p = (j + ec + 1) % KD
                        nc.tensor.matmul(ps[:], wt[:, p], src[:, p],
                                         start=(j == 0), stop=(j == KD - 1))
                nc.scalar.activation(dst[:, ec % 8], ps[:], AF.Identity,
                                     bias=bias_t[:, ec:ec + 1], scale=inv_t)

        def v_proj_piece(wvt, inv_t, src8, vdst, half, tch):
            """One (half, tch) fp8 DR group of a v projection."""
            ps = mmp.tile([128, 512], f32, tag="mm", name="vps")
            if proj_dr:
                for j in range(KD // 2):
                    p = j
                    nc.tensor.matmul(
                        ps[:],
                        src8[:, 2 * p:2 * p + 2, 128 * tch:128 * tch + 128],
                        wvt[:, 2 * p:2 * p + 2, :],
                        start=(j == 0), stop=(j == KD // 2 - 1), perf_mode=DR)
            else:
                for j in range(KD):
                    p = (j + tch) % KD
                    nc.tensor.matmul(
                        ps[:], src8[:, p, 128 * tch:128 * tch + 128],
                        wvt[:, p], start=(j == 0), stop=(j == KD - 1))
            dst = vdst[:, tch, 520 * half:520 * half + 520]
            dst = dst.rearrange("p (h f) -> p h f", f=65)[:, :, 0:64]
            nc.scalar.activation(
                dst, ps[:].rearrange("p (h f) -> p h f", f=64),
                AF.Identity, scale=inv_t)

        def v_ones(vdst):
            nc.vector.memset(vdst[:, :, 64::65], 1.0)

        def attention(qT, kT, v8, onT8, banded, filler=None):
            """Scores fp16, exp->fp8 (x eb for banded), attnV fp8 DR."""
            for hp in range(H // 2):
                es = {0: [], 1: []}
                for par in (0, 1):
                    q = qT[64 * par:64 * par + 64, hp, :]
                    for cp in (0, 1):
                        sc = scorep.tile([128, 2, TM], f32, tag="sc",
                                         name="sc")
                        for i in (0, 1):
                            c = 2 * cp + i
                            k = kT[64 * par:64 * par + 64, hp,
                                   128 * c:128 * c + 128]
                            if banded:
                                u = U0[c]
                                nc.tensor.matmul(sc[:, i, 0:UW], k,
                                                 q[:, u:u + UW],
                                                 start=True, stop=True)
                            else:
                                nc.tensor.matmul(sc[:, i, :], k, q,
                                                 start=True, stop=True)
                        e = expp.tile([128, 2, TM], fp8, tag="e", name="e")
                        if banded:
                            nc.scalar.activation(e[:, :, 0:UW],
                                                 sc[:, :, 0:UW], AF.Exp)
                            nc.vector.tensor_tensor(
                                e[:, :, 0:UW], e[:, :, 0:UW],
                                ebt[:, 2 * cp:2 * cp + 2, :], OP.mult)
                        else:
                            nc.scalar.activation(e[:], sc[:], AF.Exp)
                        es[par].append(e)
                for par in (0, 1):
                    h = 2 * hp + par
                    op_ = mmp.tile([65, TM], f32, tag="mm", name="op")
                    if attnv_dr:
                        for cp in (0, 1):
                            stat = v8[:, 2 * cp:2 * cp + 2,
                                      65 * h:65 * h + 65]
                            e = es[par][cp]
                            if banded:
                                u = U0[2 * cp]
                                nc.tensor.matmul(op_[:, u:u + UW], stat,
                                                 e[:, :, 0:UW],
                                                 start=(cp == 0),
                                                 stop=(cp == 1), perf_mode=DR)
                            else:
                                nc.tensor.matmul(op_[:], stat, e[:],
                                                 start=(cp == 0),
                                                 stop=(cp == 1), perf_mode=DR)
                    else:
                        for c in range(4):
                            stat = v8[:, c, 65 * h:65 * h + 65]
                            e = es[par][c // 2]
                            if banded:
                                u = U0[c]
                                nc.tensor.matmul(op_[:, u:u + UW], stat,
                                                 e[:, c % 2, 0:UW],
                                                 start=(c == 0), stop=(c == 3))
                            else:
                                nc.tensor.matmul(op_[:], stat, e[:, c % 2, :],
                                                 start=(c == 0), stop=(c == 3))
                    rr = smrp.tile([1, TM], f32r, tag="smr", name="rr")
                    with nc.allow_low_precision(reason="softmax denom recip"):
                        nc.vector.reciprocal(rr[:], op_[64:65, :])
                    bc = mmp.tile([64, TM], f32, tag="mm", name="bct")
                    nc.tensor.matmul(bc[:], ones_r[0:1, 0:64], rr[:],
                                     start=True, stop=True)
                    bcs = bcp.tile([64, TM], f16, tag="bcs", name="bcs")
                    nc.scalar.copy(bcs[:], bc[:])
                    if par == 0:
                        nc.vector.tensor_tensor(onT8[0:64, hp, :],
                                                op_[0:64, :], bcs[:], OP.mult)
                    else:
                        sc8 = scrp.tile([64, TM], fp8, tag="shift", name="sc8")
                        nc.vector.tensor_tensor(sc8[:], op_[0:64, :], bcs[:],
                                                OP.mult)
                        nc.sync.dma_start(onT8[64:128, hp, :], sc8[:])
                if filler is not None:
                    filler(hp)

        def out_proj_res_ln(wdram, bias_t, inv_t, onT8, resT, g_t, bt_t,
                            dst16, dst8):
            xres = t16("xres")
            for ec in range(8):
                wt = wp.tile([128, KD, 128], fp8, tag="w8", name="wot")
                nc.sync.dma_start(wt[:], wdram[ec])
                ps = mmp.tile([128, TM], f32, tag="mm", name="ops")
                for j in range(KD // 2):
                    p = j
                    nc.tensor.matmul(ps[:], wt[:, 2 * p:2 * p + 2, :],
                                     onT8[:, 2 * p:2 * p + 2, :],
                                     start=(j == 0), stop=(j == KD // 2 - 1),
                                     perf_mode=DR)
                if bz:
                    nc.vector.scalar_tensor_tensor(
                        xres[:, ec], ps[:], inv_t, resT[:, ec],
                        OP.mult, OP.add)
                else:
                    tb = tmpp.tile([128, TM], f16, tag="t1", name="tb")
                    nc.vector.scalar_tensor_tensor(
                        tb[:], ps[:], inv_t, bias_t[:, ec:ec + 1],
                        OP.mult, OP.add)
                    nc.vector.tensor_tensor(xres[:, ec], tb[:], resT[:, ec],
                                            OP.add)
            ln(xres, g_t, bt_t, dst16, dst8)

        # --- main loop over the core's batch elems -------------------------
        pending = {}   # b -> (x8, v8) prepared by previous b's CA filler
        for b in range(BPC):
            if b in pending:
                x8, v8 = pending.pop(b)
            else:
                x8 = t8("x8")
                nc.sync.dma_start(
                    x8[:], xin8[b].rearrange("k p t -> p k t"))
                v8 = None
            xT = t16("xT")
            for hh in range(2):
                nc.sync.dma_start(
                    xT[:, 4 * hh:4 * hh + 4, :],
                    xin16[b, 4 * hh:4 * hh + 4].rearrange("k p t -> p k t"))

            # A: SA q/k projections (k first: scores stationary)
            qT = t16("qT")
            kT = t16("kT")
            qk_proj(wqk_sa, bqk_sa_t, inv_qk_sa, (x8, x8), qT, kT,
                    list(range(8, 16)) + list(range(8)))
            # SA v projection (b=0 only; later b's were prefetched by the
            # previous b's CA-attention filler)
            if v8 is None:
                v8 = vp.tile([128, TCH, H * 65], fp8, tag="v8", name="v8")
                v_ones(v8)
                for half in range(2):
                    wvt = wvp.tile([128, KD, 512], fp8, tag="wv", name="wvt")
                    nc.sync.dma_start(wvt[:], wv_sa[half])
                    for tch in range(TCH):
                        v_proj_piece(wvt, inv_v_sa, x8, v8, half, tch)

            # B: audio fp8 load + CA k/v interleaved into SA attention
            a8 = t8("a8")
            nc.sync.dma_start(a8[:], ain8[b].rearrange("k p t -> p k t"))
            qT2 = t16("qT2")
            kT2 = t16("kT2")
            v28 = vp.tile([128, TCH, H * 65], fp8, tag="v8", name="v28")
            v_ones(v28)
            wvt2_holder = {}

            def sa_filler(hp):
                if hp is None:
                    return
                # one CA-k ec group + one CA-v piece per SA hp
                ec = 8 + hp
                wt = wp.tile([128, KD, 128], fp8, tag="w8", name="wqkt2")
                nc.sync.dma_start(wt[:], wqk_ca[ec])
                ps = mmp.tile([128, TM], f32, tag="mm", name="qkps2")
                for j in range(KD // 2):
                    p = j
                    nc.tensor.matmul(ps[:], wt[:, 2 * p:2 * p + 2, :],
                                     a8[:, 2 * p:2 * p + 2, :],
                                     start=(j == 0), stop=(j == KD // 2 - 1),
                                     perf_mode=DR)
                nc.scalar.activation(kT2[:, hp], ps[:], AF.Identity,
                                     bias=bqk_ca_t[:, ec:ec + 1],
                                     scale=inv_qk_ca)
                half, tch = hp // 4, hp % 4
                if tch == 0:
                    wvt2 = wvp.tile([128, KD, 512], fp8, tag="wv",
                                    name="wvt2")
                    nc.sync.dma_start(wvt2[:], wv_ca[half])
                    wvt2_holder[half] = wvt2
                v_proj_piece(wvt2_holder[half], inv_v_ca, a8, v28, half, tch)

            onT8 = t8("onT8")
            attention(qT, kT, v8, onT8, banded=False,
                      filler=None if nofill else sa_filler)
            if nofill:
                for hp in range(8):
                    sa_filler(hp)

            # C: SA out-proj + residual + LN1
            x1T = t16("x1T")
            x1T8 = t8("x1T8")
            out_proj_res_ln(wo_sa, bo_sa_t, inv_o_sa, onT8, xT,
                            n1g_t, n1b_t, x1T, x1T8)

            # D: CA q projection from x1
            qk_proj(wqk_ca, bqk_ca_t, inv_qk_ca, (x1T8, None), qT2, kT2,
                    list(range(8)))

            # E: CA attention (banded), next-b v_proj interleaved
            onT28 = t8("onT28")
            if b + 1 < BPC:
                nx8 = t8("x8")
                nc.sync.dma_start(
                    nx8[:], xin8[b + 1].rearrange("k p t -> p k t"))
                nv8 = vp.tile([128, TCH, H * 65], fp8, tag="v8", name="nv8")
                v_ones(nv8)
                pending[b + 1] = (nx8, nv8)
                nwv_holder = {}

                def ca_filler(hp):
                    half, tch = hp // 4, hp % 4
                    if tch == 0:
                        nwvt = wvp.tile([128, KD, 512], fp8, tag="wv",
                                        name="nwvt")
                        nc.sync.dma_start(nwvt[:], wv_sa[half])
                        nwv_holder[half] = nwvt
                    v_proj_piece(nwv_holder[half], inv_v_sa, nx8, nv8,
                                 half, tch)
            else:
                ca_filler = None
            attention(qT2, kT2, v28, onT28, banded=True,
                      filler=None if nofill else ca_filler)
            if nofill and ca_filler is not None:
                for hp in range(8):
                    ca_filler(hp)

            # F: CA out-proj (tanh(gate) folded on host) + residual + LNc
            x2T = t16("x2T")
            x2T8 = t8("x2T8") if lin1_fp8 else None
            out_proj_res_ln(wo_ca, bo_ca_t, inv_o_ca, onT28, x1T,
                            ncg_t, ncb_t, x2T, x2T8)

            # G: lin1 + gelu -> hq
            hq = hqp.tile([128, KF, TM], fp8 if lin2_fp8 else f16,
                          tag="hq", name="hq")
            for fc in range(KF):
                wpool = wp if lin1_fp8 else w16p
                wt = wpool.tile([128, KD, 128], fp8 if lin1_fp8 else f16,
                                tag="w8" if lin1_fp8 else "w16", name="w1t")
                nc.sync.dma_start(wt[:], w1[fc])
                ps = mmp.tile([128, TM], f32, tag="mm", name="hps")
                if lin1_fp8:
                    for j in range(KD // 2):
                        p = j
                        nc.tensor.matmul(ps[:], wt[:, 2 * p:2 * p + 2, :],
                                         x2T8[:, 2 * p:2 * p + 2, :],
                                         start=(j == 0),
                                         stop=(j == KD // 2 - 1),
                                         perf_mode=DR)
                else:
                    for j in range(KD):
                        k = (j + fc) % KD
                        nc.tensor.matmul(ps[:], wt[:, k], x2T[:, k],
                                         start=(j == 0), stop=(j == KD - 1))
                nc.scalar.activation(hq[:, fc], ps[:], AF.Gelu,
                                     bias=b1_t[:, fc:fc + 1], scale=inv_1)

            # H: lin2 + residual + LN2 -> out
            xres2 = t16("xres2")
            for ec in range(8):
                wt = w2p.tile([128, KF, 128], fp8 if lin2_fp8 else f16,
                              tag="w2", name="w2t")
                nc.sync.dma_start(wt[:], w2[ec])
                ps = mmp.tile([128, TM], f32, tag="mm", name="fps")
                if lin2_fp8:
                    for j in range(KF // 2):
                        p = j
                        nc.tensor.matmul(ps[:], wt[:, 2 * p:2 * p + 2, :],
                                         hq[:, 2 * p:2 * p + 2, :],
                                         start=(j == 0),
                                         stop=(j == KF // 2 - 1),
                                         perf_mode=DR)
                else:
                    for j in range(KF):
                        p = (j + 4 * ec) % KF
                        nc.tensor.matmul(ps[:], wt[:, p], hq[:, p],
                                         start=(j == 0), stop=(j == KF - 1))
                if bz:
                    nc.vector.scalar_tensor_tensor(
                        xres2[:, ec], ps[:], inv_2, x2T[:, ec],
                        OP.mult, OP.add)
                else:
                    tb = tmpp.tile([128, TM], f16, tag="t1", name="tb2")
                    nc.vector.scalar_tensor_tensor(
                        tb[:], ps[:], inv_2, b2_t[:, ec:ec + 1],
                        OP.mult, OP.add)
                    nc.vector.tensor_tensor(xres2[:, ec], tb[:], x2T[:, ec],
                                            OP.add)
            outT = t16("outT")
            ln(xres2, n2g_t, n2b_t, outT, None)
            for hh in range(2):
                nc.sync.dma_start(
                    out[b, 4 * hh:4 * hh + 4].rearrange("k p t -> p k t"),
                    outT[:, 4 * hh:4 * hh + 4, :])

    return nc


def _get_module(cfg=None):
    if cfg is None:
        cfg = _CACHE.get("last_cfg",
                         (True, True, False, False, True, False, True))
    _CACHE["last_cfg"] = cfg
    key = ("nc", cfg)
    if key not in _CACHE:
        _install_patches()
        _CACHE[key] = _build_module(*cfg)
    return _CACHE[key]


# ---------------------------------------------------------------------------
# Host-side prep + execution
# ---------------------------------------------------------------------------
def _beat_bias(beats):
    beats = np.asarray(beats).astype(np.int64).ravel()
    bias = np.zeros(TA, np.float32)
    l_idx = np.where(beats > 0, beats - 1, 0)
    l_val = np.where(beats > 0, BW * 0.5, 0.0).astype(np.float32)
    r_idx = np.where(beats < TA - 1, beats + 1, TA - 1)
    r_val = np.where(beats < TA - 1, BW * 0.5, 0.0).astype(np.float32)
    np.maximum.at(bias, l_idx, l_val)
    np.maximum.at(bias, r_idx, r_val)
    np.maximum.at(bias, beats, np.float32(BW))
    return bias


def _temporal_bias():
    scale = (TA - 1) / (TM - 1)
    audio_pos = np.arange(TM, dtype=np.float32) * scale
    diff = audio_pos[:, None] - np.arange(TA, dtype=np.float32)[None, :]
    return (-(diff ** 2) / (2.0 * SIGMA ** 2)).astype(np.float32)


def _chunk_w(w, n_out_chunks, n_in_chunks, dt):
    # w: [E, Dk] row-major -> [ec, p(in), kc, j(out)]
    E, Dk = w.shape
    return np.ascontiguousarray(
        w.reshape(n_out_chunks, E // n_out_chunks, n_in_chunks, Dk // n_in_chunks)
        .transpose(0, 3, 2, 1).astype(dt))


def _pp(vec):
    # [n*128] -> [128, n] per-partition layout
    v = np.asarray(vec, np.float32).reshape(-1, 128)
    return np.ascontiguousarray(v.T)


def _pow2scale(w, target=192.0):
    m = float(np.abs(w).max())
    if m == 0:
        return 1.0
    return float(2.0 ** np.floor(np.log2(target / m)))


def kernel(**inputs):
    import ml_dtypes
    from concourse.bass_utils import run_bass_kernel_spmd

    fp8 = ml_dtypes.float8_e4m3
    f16 = np.float16
    f32 = np.float32

    src = np.asarray(inputs["src"], f32)
    audio = np.asarray(inputs["audio_memory"], f32)
    beats = inputs["beat_frames"]

    ln_id = not (
        np.any(np.asarray(inputs["n1_b"])) or np.any(np.asarray(inputs["nc_b"]))
        or np.any(np.asarray(inputs["n2_b"]))
        or np.any(np.asarray(inputs["n1_g"]) != 1.0)
        or np.any(np.asarray(inputs["nc_g"]) != 1.0)
        or np.any(np.asarray(inputs["n2_g"]) != 1.0))

    sa_in_w = np.asarray(inputs["sa_in_w"], f32)
    sa_in_b = np.asarray(inputs["sa_in_b"], f32)
    sa_out_w = np.asarray(inputs["sa_out_w"], f32)
    sa_out_b = np.asarray(inputs["sa_out_b"], f32)
    ca_in_w = np.asarray(inputs["ca_in_w"], f32)
    ca_in_b = np.asarray(inputs["ca_in_b"], f32)
    ca_out_w = np.asarray(inputs["ca_out_w"], f32)
    ca_out_b = np.asarray(inputs["ca_out_b"], f32)
    gate = float(np.asarray(inputs["gate"]))
    tg = float(np.tanh(gate))
    lin1_b = np.asarray(inputs["lin1_b"], f32)
    lin2_b = np.asarray(inputs["lin2_b"], f32)

    # SA: fold 1/8 score scale into q weights+bias; v-bias into out-proj bias.
    wqk_sa_eff = np.concatenate([sa_in_w[:D] / 8.0, sa_in_w[D:2 * D]], axis=0)
    bqk_sa_eff = np.concatenate([sa_in_b[:D] / 8.0, sa_in_b[D:2 * D]])
    bo_sa_eff = sa_out_b + sa_out_w @ sa_in_b[2 * D:]
    wqk_ca_eff = np.concatenate([ca_in_w[:D] / 8.0, ca_in_w[D:2 * D]], axis=0)
    bqk_ca_eff = np.concatenate([ca_in_b[:D] / 8.0, ca_in_b[D:2 * D]])
    wo_ca_eff = tg * ca_out_w
    bo_ca_eff = tg * (ca_out_b + ca_out_w @ ca_in_b[2 * D:])

    bz = not (np.any(bqk_sa_eff) or np.any(bo_sa_eff) or np.any(bqk_ca_eff)
              or np.any(bo_ca_eff) or np.any(lin2_b))
    lin1_fp8 = os.environ.get("K_LIN1_FP8", "0") == "1"
    lin2_fp8 = os.environ.get("K_LIN2_FP8", "0") == "1"
    attnv_dr = os.environ.get("K_ATTNV_DR", "1") == "1"
    proj_dr = os.environ.get("K_PROJ_DR", "0") == "1"
    nofill = os.environ.get("K_NOFILL", "1") == "1"
    nc = _get_module((ln_id, bz, lin1_fp8, lin2_fp8, attnv_dr, proj_dr,
                      nofill))

    # feature-major activations: [B, KD, 128, T]
    xfm = src.transpose(1, 2, 0).reshape(B, KD, 128, TM)
    afm = audio.transpose(1, 2, 0).reshape(B, KD, 128, TA)
    xin16_all = np.ascontiguousarray(xfm.astype(f16))
    xin8_all = np.ascontiguousarray(xfm.astype(fp8))
    ain8_all = np.ascontiguousarray(afm.astype(fp8))

    # fp8 weight scaling (pow2 per tensor)
    s_qk_sa = _pow2scale(wqk_sa_eff)
    s_v_sa = _pow2scale(sa_in_w[2 * D:])
    s_o_sa = _pow2scale(sa_out_w)
    s_qk_ca = _pow2scale(wqk_ca_eff)
    s_v_ca = _pow2scale(ca_in_w[2 * D:])
    s_o_ca = _pow2scale(wo_ca_eff)
    w1_f = np.asarray(inputs["lin1_w"], f32)
    w2_f = np.asarray(inputs["lin2_w"], f32)
    s_1 = _pow2scale(w1_f) if lin1_fp8 else 1.0
    s_2 = _pow2scale(w2_f) if lin2_fp8 else 1.0

    # exp(bias) tiles: [tk-part 128, chunk 4, tq-local 384], 0 outside band
    bias = _temporal_bias() + _beat_bias(beats)[None, :]  # [tq, tk]
    bT = bias.T  # [tk, tq]
    ebt = np.zeros((128, 4, UW), f32)
    for c in range(4):
        lo, w = TQR[c]
        u = U0[c]
        blk = np.exp(bT[128 * c:128 * c + 128, lo:lo + w])
        ebt[:, c, lo - u:lo - u + w] = blk
    ebt = ebt.astype(f16)

    # v half-major weight layout: [half][in-part 128, kd, 512 out]
    def _wv_prep(wv, s):
        a = _chunk_w(wv * s, 2, KD, fp8)  # [2, 128, KD, 512]
        return np.ascontiguousarray(a)

    weights = {
        "wqk_sa": _chunk_w(wqk_sa_eff * s_qk_sa, 16, KD, fp8),
        "wv_sa": _wv_prep(sa_in_w[2 * D:], s_v_sa),
        "wo_sa": _chunk_w(sa_out_w * s_o_sa, 8, KD, fp8),
        "wqk_ca": _chunk_w(wqk_ca_eff * s_qk_ca, 16, KD, fp8),
        "wv_ca": _wv_prep(ca_in_w[2 * D:], s_v_ca),
        "wo_ca": _chunk_w(wo_ca_eff * s_o_ca, 8, KD, fp8),
        "w1": _chunk_w(w1_f * s_1, KF, KD, fp8 if lin1_fp8 else f16),
        "w2": _chunk_w(w2_f * s_2, 8, KF, fp8 if lin2_fp8 else f16),
        "ppb": np.ascontiguousarray(np.concatenate([
            _pp(bqk_sa_eff),
            _pp(bo_sa_eff), _pp(bqk_ca_eff),
            _pp(bo_ca_eff),
            _pp(lin1_b),
            _pp(lin2_b),
            _pp(np.asarray(inputs["n1_g"], f32)),
            _pp(np.asarray(inputs["n1_b"], f32)),
            _pp(np.asarray(inputs["nc_g"], f32)),
            _pp(np.asarray(inputs["nc_b"], f32)),
            _pp(np.asarray(inputs["n2_g"], f32)),
            _pp(np.asarray(inputs["n2_b"], f32)),
            np.full((128, 1), 1.0 / s_qk_sa, f32),
            np.full((128, 1), 1.0 / s_v_sa, f32),
            np.full((128, 1), 1.0 / s_o_sa, f32),
            np.full((128, 1), 1.0 / s_qk_ca, f32),
            np.full((128, 1), 1.0 / s_v_ca, f32),
            np.full((128, 1), 1.0 / s_o_ca, f32),
            np.full((128, 1), 1.0 / s_1, f32),
            np.full((128, 1), 1.0 / s_2, f32),
        ], axis=1)),
        "eb": ebt,
        "onescol": np.ones((128, 1), f32),
        "onesrow": np.ones((1, 128), f32),
    }

    in_maps = []
    for c in range(NCORES):
        m = dict(weights)
        m["xin16"] = np.ascontiguousarray(xin16_all[BPC * c:BPC * (c + 1)])
        m["xin8"] = np.ascontiguousarray(xin8_all[BPC * c:BPC * (c + 1)])
        m["ain8"] = np.ascontiguousarray(ain8_all[BPC * c:BPC * (c + 1)])
        in_maps.append(m)

    res = run_bass_kernel_spmd(nc, in_maps, core_ids=list(range(NCORES)))
    outs = [r["out"] for r in res.results]  # each [BPC, KD, 128, TM] f16
    full = np.concatenate(outs, axis=0).astype(np.float32)
    return np.ascontiguousarray(
        full.reshape(B, D, TM).transpose(2, 0, 1))
